# revision 31
# baseline (speedup 1.0000x reference)
"""AxialAttention TRN2 kernel.

Sharding: 8 cores = 4 batches x 2 head-groups (4 heads each). Each core:
  phase 1: qkv projection (fp32r matmuls, x stationary) -> s-major qkv [S, 768]
           bf16 written to DRAM in BOTH row-major (h,w) and col-major (w,h)
           pixel orders; bias folded in as a K=1 matmul. Row-major copy is
           split into 4 quarter tiles so the row-attention branch can start
           while the projection is still running.
  phase 2: row + col axial attention (bf16): identical code per branch reading
           the matching qkv layout in 8-tile chunks. Scores via k^T q
           (d contracted), softmax without max-subtraction (scores bounded),
           per-query sums via a ones-matmul broadcast, 1/Z on DVE,
           normalization fused into the drains. Row branch writes O, col
           branch adds (with pre-normalized P).
  phase 3: output projection (bf16) of row+col sum; bias on group-0 cores only.
Host: per-batch sum of the two head-group partial outputs.

DMA routing: nc.sync carries ONLY transpose DMAs (XBAR transpose<->copy
interleave corrupts data on this HW), nc.scalar the phase-1 streams,
nc.gpsimd (SWDGE) the v/out streams.
"""

import numpy as np
import ml_dtypes
from contextlib import ExitStack

import concourse.bass as bass
import concourse.bacc as bacc
import concourse.tile as tile
from concourse import mybir
from concourse.bass_utils import run_bass_kernel_spmd

C = 512          # channels
H = 128          # height
W = 128          # width
S = H * W        # 16384 pixels
NH = 8           # total heads
D = 64           # head dim
NHC = 4          # heads per core
GC = NHC * D     # 256 group channels (q or k or v)
QKV = 3 * GC     # 768 projected channels per core
CT = C // 128    # 4 contraction tiles
ST = S // 128    # 128 spatial tiles
NQ = 4           # qkvB quarter tiles
SCALE = 1.0 / np.sqrt(D)

F32 = mybir.dt.float32
F32R = mybir.dt.float32r
BF16 = mybir.dt.bfloat16
EXP = mybir.ActivationFunctionType.Exp
IDENT = mybir.ActivationFunctionType.Identity
ADD = mybir.AluOpType.add
MULT = mybir.AluOpType.mult

_CACHED_NC = None


def build_nc(debug_dump=False, reps=1):
    nc = bacc.Bacc()
    x_in = nc.dram_tensor("x", [C, S], F32R, kind="ExternalInput")
    wqkvT = nc.dram_tensor("wqkvT", [C, QKV], F32R, kind="ExternalInput")
    bqkv = nc.dram_tensor("bqkv", [1, QKV], F32, kind="ExternalInput")
    woutT = nc.dram_tensor("woutT", [GC, C], BF16, kind="ExternalInput")
    bout = nc.dram_tensor("bout", [128, CT], F32, kind="ExternalInput")
    out = nc.dram_tensor("out", [C, S], F32, kind="ExternalOutput")
    if debug_dump:
        dbg_qkv = nc.dram_tensor("dbg_qkv", [S, QKV], BF16, kind="ExternalOutput")
        dbg_O = nc.dram_tensor("dbg_O", [2, 128, S], BF16, kind="ExternalOutput")

    with tile.TileContext(nc) as tc, ExitStack() as ctx:
        persist = ctx.enter_context(tc.tile_pool(name="persist", bufs=1))
        dram = ctx.enter_context(tc.tile_pool(name="dram", bufs=1, space="DRAM"))

        # --- persistent tiles ---
        w_sb = persist.tile([128, CT, QKV], F32R, tag="w_sb")
        nc.scalar.dma_start(
            out=w_sb, in_=wqkvT.ap().rearrange("(t p) o -> p t o", p=128)
        )
        bias_bc = persist.tile([128, QKV], F32, tag="bias_bc")
        bq_ap = bqkv.ap()
        nc.scalar.dma_start(
            out=bias_bc,
            in_=bass.AP(tensor=bq_ap.tensor, offset=0, ap=[[0, 128], [1, QKV]]),
        )
        wout_sb = persist.tile([128, 2, C], BF16, tag="wout_sb")
        nc.scalar.dma_start(
            out=wout_sb, in_=woutT.ap().rearrange("(t p) o -> p t o", p=128)
        )
        boutv = persist.tile([128, CT], F32, tag="boutv")
        nc.scalar.dma_start(out=boutv, in_=bout.ap())
        ones_sb = persist.tile([128, 128], BF16, tag="ones_sb")
        nc.vector.memset(ones_sb, 1.0)

        O_sb = [
            persist.tile([128, S], BF16, tag=f"O{i}", name=f"O{i}") for i in range(2)
        ]
        Oc_sb = [
            persist.tile([128, S], BF16, tag=f"Oc{i}", name=f"Oc{i}") for i in range(2)
        ]

        # row-major copy in quarters (pixel order s = h*W + w)
        qkvQ = [
            dram.tile([S // NQ, QKV], BF16, tag=f"qkvQ{i}", name=f"qkvQ{i}")
            for i in range(NQ)
        ]
        qkvB2 = dram.tile([S, 512], BF16)   # q,k only; pixel order s' = w*H + h

        for _rep in range(reps):
            build_body(nc, tc, x_in, w_sb, bias_bc, wout_sb, boutv,
                       ones_sb, O_sb, Oc_sb, qkvQ, qkvB2, out)

        if debug_dump:
            for qi in range(NQ):
                nc.scalar.dma_start(
                    out=dbg_qkv[qi * (S // NQ) : (qi + 1) * (S // NQ), :],
                    in_=qkvQ[qi][:],
                )
            for i in range(2):
                nc.scalar.dma_start(out=dbg_O.ap()[i], in_=O_sb[i])

    nc.finalize()
    return nc


def build_body(nc, tc, x_in, w_sb, bias_bc, wout_sb, boutv, ones_sb,
               O_sb, Oc_sb, qkvQ, qkvB2, out):
    # ---------- phase 1: qkv projection (x stationary, s-major out) ----------
    x_r = x_in.ap().rearrange("(t p) s -> p t s", p=128)
    with (
        tc.tile_pool(name="p1x", bufs=3) as xpool,
        tc.tile_pool(name="p1ps", bufs=3, space="PSUM") as pspool,
        tc.tile_pool(name="p1o", bufs=3) as opool,
    ):
        for sg in range(ST // 4):  # groups of 4 s-tiles (one h-quad)
            xg = xpool.tile([128, CT, 512], F32R)
            nc.scalar.dma_start(out=xg, in_=x_r[:, :, sg * 512 : (sg + 1) * 512])
            qt4 = opool.tile([128, 4, QKV], BF16)
            for i in range(4):
                ps = pspool.tile([128, QKV], F32)
                for ct in range(CT):
                    lhsT = xg[:, ct, i * 128 : (i + 1) * 128]
                    nc.tensor.matmul(
                        out=ps[:, 0:512], lhsT=lhsT, rhs=w_sb[:, ct, 0:512],
                        start=(ct == 0), stop=(ct == CT - 1),
                    )
                    nc.tensor.matmul(
                        out=ps[:, 512:QKV], lhsT=lhsT, rhs=w_sb[:, ct, 512:QKV],
                        start=(ct == 0), stop=(ct == CT - 1),
                    )
                nc.vector.tensor_tensor(out=qt4[:, i, :], in0=ps, in1=bias_bc, op=ADD)
            # rows (sg*4+i)*128 + w of the row-major copy -> quarter sg//8
            qi, sgq = sg // 8, sg % 8
            dstA = qkvQ[qi][sgq * 512 : (sgq + 1) * 512, :].rearrange(
                "(i p) o -> p i o", p=128
            )
            nc.scalar.dma_start(out=dstA, in_=qt4)
            # rows w*128 + (sg*4+i) of the col-major q,k copy
            dstB = qkvB2[:].rearrange("(p i) o -> p i o", i=ST)[
                :, sg * 4 : (sg + 1) * 4, :
            ]
            nc.scalar.dma_start(out=dstB, in_=qt4[:, :, 0:512])

    # ---------- phase 2: axial attention (8-tile chunks) ----------
    with (
        tc.tile_pool(name="a_qt", bufs=3) as qtpool,
        tc.tile_pool(name="a_kt", bufs=3) as ktpool,
        tc.tile_pool(name="a_vt", bufs=2) as vtpool,
        tc.tile_pool(name="a_p", bufs=3) as ppool,
        tc.tile_pool(name="a_rz", bufs=2) as rzpool,
        tc.tile_pool(name="a_psS", bufs=2, space="PSUM") as psumS,
        tc.tile_pool(name="a_psZ", bufs=1, space="PSUM") as psumZ,
        tc.tile_pool(name="a_psO", bufs=1, space="PSUM") as psumO,
    ):
        for branch in range(2):  # 0 = row (writes O), 1 = col (adds into O)
            for tg in range(ST // 8):  # chunks of 8 attention tiles
                if branch == 0:
                    rows = qkvQ[tg // 4][(tg % 4) * 1024 : (tg % 4 + 1) * 1024, :]
                else:
                    rows = qkvB2[tg * 1024 : (tg + 1) * 1024, :]
                vt8 = vtpool.tile([128, 8, 256], BF16)
                if branch == 0:
                    nc.gpsimd.dma_start(
                        out=vt8,
                        in_=rows[:, 512:768].rearrange("(i p) o -> p i o", p=128),
                    )
                else:
                    # v for col tiles w = tg*8+i: element (g=h, i, d) lives in
                    # quarter qi at row (h - 32*qi)*128 + w, col 512 + d
                    for qi in range(NQ):
                        src = qkvQ[qi][:].rearrange(
                            "(h w) o -> h w o", w=W
                        )[:, tg * 8 : (tg + 1) * 8, 512:768]
                        nc.gpsimd.dma_start(
                            out=vt8[32 * qi : 32 * (qi + 1), :, :], in_=src
                        )
                for hp in range(2):  # head pair
                    q8 = qtpool.tile([128, 1024], BF16)
                    nc.sync.dma_start_transpose(
                        out=q8, in_=rows[:, hp * 128 : (hp + 1) * 128]
                    )
                    k8 = ktpool.tile([128, 1024], BF16)
                    nc.sync.dma_start_transpose(
                        out=k8, in_=rows[:, 256 + hp * 128 : 256 + (hp + 1) * 128]
                    )
                    # PV output / Z of both heads stacked on partition halves
                    psO = [psumO.tile([128, 512], F32, name=f"psO{j}") for j in range(2)]
                    psZ = psumZ.tile([128, 1024], F32)
                    for hl in range(2):  # head within pair
                        r0, r1 = hl * 64, (hl + 1) * 64
                        psS = psumS.tile([128, 1024], F32)
                        for i in range(8):
                            nc.tensor.matmul(
                                out=psS[:, i * 128 : (i + 1) * 128],
                                lhsT=k8[r0:r1, i * 128 : (i + 1) * 128],
                                rhs=q8[r0:r1, i * 128 : (i + 1) * 128],
                                start=True, stop=True,
                            )
                        pch = ppool.tile([128, 1024], BF16)
                        nc.scalar.activation(
                            out=pch, in_=psS, func=EXP, scale=float(SCALE)
                        )
                        for j in range(2):
                            nc.tensor.matmul(
                                out=psZ[r0:r1, j * 512 : (j + 1) * 512],
                                lhsT=ones_sb[:, 0:64],
                                rhs=pch[:, j * 512 : (j + 1) * 512],
                                start=True, stop=True,
                            )
                        for j in range(2):  # half-chunks of 4 tiles
                            for i in range(4):
                                ii = j * 4 + i
                                nc.tensor.matmul(
                                    out=psO[j][r0:r1, i * 128 : (i + 1) * 128],
                                    lhsT=vt8[:, ii, hp * 128 + r0 : hp * 128 + r1],
                                    rhs=pch[:, ii * 128 : (ii + 1) * 128],
                                    start=True, stop=True,
                                )
                    rzs = rzpool.tile([128, 1024], F32)  # hl-stacked 1/Z
                    nc.vector.reciprocal_approx_fast(out=rzs, in_=psZ)
                    for j in range(2):  # drain both heads at once
                        t0 = tg * 8 + j * 4  # first tile of this half
                        if branch == 0:
                            nc.vector.tensor_tensor(
                                out=O_sb[hp][:, t0 * 128 : t0 * 128 + 512],
                                in0=psO[j],
                                in1=rzs[:, j * 512 : (j + 1) * 512],
                                op=MULT,
                            )
                        else:
                            dst = Oc_sb[hp][:, :].rearrange(
                                "p (h w) -> p h w", w=W
                            )[:, :, t0 : t0 + 4]
                            nc.vector.tensor_tensor(
                                out=dst,
                                in0=psO[j].rearrange("p (w h) -> p h w", w=4),
                                in1=rzs[:, j * 512 : (j + 1) * 512].rearrange(
                                    "p (w h) -> p h w", w=4
                                ),
                                op=MULT,
                            )

    # ---------- merge col branch into O, then phase 3 ----------
    for hp in range(2):
        for q in range(4):
            nc.vector.tensor_tensor(
                out=O_sb[hp][:, q * 4096 : (q + 1) * 4096],
                in0=O_sb[hp][:, q * 4096 : (q + 1) * 4096],
                in1=Oc_sb[hp][:, q * 4096 : (q + 1) * 4096],
                op=ADD,
            )

    # ---------- phase 3: output projection ----------
    out_r = out.ap().rearrange("(t p) s -> p t s", p=128)
    with (
        tc.tile_pool(name="f_ps", bufs=3, space="PSUM") as psumF,
        tc.tile_pool(name="f_o", bufs=3) as fpool,
    ):
        for ch in range(S // 512):
            of4 = fpool.tile([128, CT, 512], F32)
            for ot in range(CT):
                psF = psumF.tile([128, 512], F32)
                for hp in range(2):
                    nc.tensor.matmul(
                        out=psF,
                        lhsT=wout_sb[:, hp, ot * 128 : (ot + 1) * 128],
                        rhs=O_sb[hp][:, ch * 512 : (ch + 1) * 512],
                        start=(hp == 0), stop=(hp == 1),
                    )
                if ot < 2:
                    nc.scalar.activation(
                        out=of4[:, ot, :], in_=psF, func=IDENT,
                        bias=boutv[:, ot : ot + 1], scale=1.0,
                    )
                else:
                    nc.vector.tensor_scalar_add(
                        out=of4[:, ot, :], in0=psF, scalar1=boutv[:, ot : ot + 1]
                    )
            nc.gpsimd.dma_start(
                out=out_r[:, :, ch * 512 : (ch + 1) * 512], in_=of4
            )


def get_nc():
    global _CACHED_NC
    if _CACHED_NC is None:
        _CACHED_NC = build_nc()
    return _CACHED_NC


def make_in_maps(x, Wqkv, bqkv, Wout, bout):
    """Per-core input dicts: core c = (b, g) with b = c // 2, g = c % 2."""
    in_maps = []
    for c in range(8):
        b, g = c // 2, c % 2
        sel = slice(256 * g, 256 * (g + 1))
        wsel = np.concatenate(
            [Wqkv[sel, :], Wqkv[512 + 256 * g : 512 + 256 * (g + 1), :],
             Wqkv[1024 + 256 * g : 1024 + 256 * (g + 1), :]], axis=0
        )  # [768, 512]
        bsel = np.concatenate(
            [bqkv[sel], bqkv[512 + 256 * g : 512 + 256 * (g + 1)],
             bqkv[1024 + 256 * g : 1024 + 256 * (g + 1)]]
        )  # [768]
        woutT = np.ascontiguousarray(Wout[:, sel].T)  # [256, 512]
        in_maps.append(
            {
                "x": np.ascontiguousarray(x[b].reshape(C, S)),
                "wqkvT": np.ascontiguousarray(wsel.T),
                "bqkv": bsel.reshape(1, QKV).copy(),
                "woutT": woutT.astype(ml_dtypes.bfloat16),
                "bout": (
                    np.ascontiguousarray(bout.reshape(CT, 128).T)
                    if g == 0
                    else np.zeros((128, CT), np.float32)
                ),
            }
        )
    return in_maps


def kernel(x, Wqkv, bqkv, Wout, bout):
    x = np.asarray(x, dtype=np.float32)
    Wqkv = np.asarray(Wqkv, dtype=np.float32)
    bqkv = np.asarray(bqkv, dtype=np.float32)
    Wout = np.asarray(Wout, dtype=np.float32)
    bout = np.asarray(bout, dtype=np.float32)

    nc = get_nc()
    in_maps = make_in_maps(x, Wqkv, bqkv, Wout, bout)
    res = run_bass_kernel_spmd(nc, in_maps, core_ids=list(range(8)))
    B = x.shape[0]
    out = np.empty((B, C, H, W), dtype=np.float32)
    for b in range(B):
        acc = res.results[2 * b]["out"] + res.results[2 * b + 1]["out"]
        out[b] = acc.reshape(C, H, W)
    return out


# revision 33
# speedup vs baseline: 114.3857x; 114.3857x over previous
"""AxialAttention TRN2 kernel.

Sharding: 8 cores = 4 batches x 2 head-groups (4 heads each). Each core:
  phase 1: qkv projection (fp32r matmuls, x stationary) -> s-major qkv [S, 768]
           bf16 written to DRAM in BOTH row-major (h,w) and col-major (w,h)
           pixel orders; bias folded in as a K=1 matmul. Row-major copy is
           split into 4 quarter tiles so the row-attention branch can start
           while the projection is still running.
  phase 2: row + col axial attention (bf16): identical code per branch reading
           the matching qkv layout in 8-tile chunks. Scores via k^T q
           (d contracted), softmax without max-subtraction (scores bounded),
           per-query sums via a ones-matmul broadcast, 1/Z on DVE,
           normalization fused into the drains. Row branch writes O, col
           branch adds (with pre-normalized P).
  phase 3: output projection (bf16) of row+col sum; bias on group-0 cores only.
Host: per-batch sum of the two head-group partial outputs.

DMA routing: nc.sync carries ONLY transpose DMAs (XBAR transpose<->copy
interleave corrupts data on this HW), nc.scalar the phase-1 streams,
nc.gpsimd (SWDGE) the v/out streams.
"""

import numpy as np
import ml_dtypes
from contextlib import ExitStack

import concourse.bass as bass
import concourse.bacc as bacc
import concourse.tile as tile
from concourse import mybir
from concourse.bass_utils import run_bass_kernel_spmd

C = 512          # channels
H = 128          # height
W = 128          # width
S = H * W        # 16384 pixels
NH = 8           # total heads
D = 64           # head dim
NHC = 4          # heads per core
GC = NHC * D     # 256 group channels (q or k or v)
QKV = 3 * GC     # 768 projected channels per core
CT = C // 128    # 4 contraction tiles
ST = S // 128    # 128 spatial tiles
NQ = 4           # qkvB quarter tiles
SCALE = 1.0 / np.sqrt(D)

F32 = mybir.dt.float32
F32R = mybir.dt.float32r
BF16 = mybir.dt.bfloat16
EXP = mybir.ActivationFunctionType.Exp
IDENT = mybir.ActivationFunctionType.Identity
ADD = mybir.AluOpType.add
MULT = mybir.AluOpType.mult

_CACHED_NC = None


def build_nc(debug_dump=False, reps=1):
    nc = bacc.Bacc()
    x_in = nc.dram_tensor("x", [C, S], F32R, kind="ExternalInput")
    wqkvT = nc.dram_tensor("wqkvT", [C, QKV], F32R, kind="ExternalInput")
    bqkv = nc.dram_tensor("bqkv", [1, QKV], F32R, kind="ExternalInput")
    ones1 = nc.dram_tensor("ones1", [1, 128], F32R, kind="ExternalInput")
    woutT = nc.dram_tensor("woutT", [GC, C], BF16, kind="ExternalInput")
    bout = nc.dram_tensor("bout", [128, CT], F32, kind="ExternalInput")
    out = nc.dram_tensor("out", [C, S], F32, kind="ExternalOutput")
    if debug_dump:
        dbg_qkv = nc.dram_tensor("dbg_qkv", [S, QKV], BF16, kind="ExternalOutput")
        dbg_O = nc.dram_tensor("dbg_O", [2, 128, S], BF16, kind="ExternalOutput")

    with tile.TileContext(nc) as tc, ExitStack() as ctx:
        persist = ctx.enter_context(tc.tile_pool(name="persist", bufs=1))
        dram = ctx.enter_context(tc.tile_pool(name="dram", bufs=1, space="DRAM"))

        # --- persistent tiles ---
        w_sb = persist.tile([128, CT, QKV], F32R, tag="w_sb")
        nc.scalar.dma_start(
            out=w_sb, in_=wqkvT.ap().rearrange("(t p) o -> p t o", p=128)
        )
        brow_sb = persist.tile([1, QKV], F32R, tag="brow_sb")
        nc.scalar.dma_start(out=brow_sb, in_=bqkv.ap())
        ones1_sb = persist.tile([1, 128], F32R, tag="ones1_sb")
        nc.scalar.dma_start(out=ones1_sb, in_=ones1.ap())
        wout_sb = persist.tile([128, 2, C], BF16, tag="wout_sb")
        nc.scalar.dma_start(
            out=wout_sb, in_=woutT.ap().rearrange("(t p) o -> p t o", p=128)
        )
        boutv = persist.tile([128, CT], F32, tag="boutv")
        nc.scalar.dma_start(out=boutv, in_=bout.ap())
        ones_sb = persist.tile([128, 128], BF16, tag="ones_sb")
        nc.vector.memset(ones_sb, 1.0)

        O_sb = [
            persist.tile([128, S], BF16, tag=f"O{i}", name=f"O{i}") for i in range(2)
        ]

        # row-major copy in quarters (pixel order s = h*W + w)
        qkvQ = [
            dram.tile([S // NQ, QKV], BF16, tag=f"qkvQ{i}", name=f"qkvQ{i}")
            for i in range(NQ)
        ]
        qkvB2 = dram.tile([S, 512], BF16)   # q,k only; pixel order s' = w*H + h

        for _rep in range(reps):
            build_body(nc, tc, x_in, w_sb, brow_sb, ones1_sb, wout_sb, boutv,
                       ones_sb, O_sb, qkvQ, qkvB2, out)

        if debug_dump:
            for qi in range(NQ):
                nc.scalar.dma_start(
                    out=dbg_qkv[qi * (S // NQ) : (qi + 1) * (S // NQ), :],
                    in_=qkvQ[qi][:],
                )
            for i in range(2):
                nc.scalar.dma_start(out=dbg_O.ap()[i], in_=O_sb[i])

    nc.finalize()
    return nc


def build_body(nc, tc, x_in, w_sb, brow_sb, ones1_sb, wout_sb, boutv, ones_sb,
               O_sb, qkvQ, qkvB2, out):
    # ---------- phase 1: qkv projection (x stationary, s-major out) ----------
    x_r = x_in.ap().rearrange("(t p) s -> p t s", p=128)
    with (
        tc.tile_pool(name="p1x", bufs=3) as xpool,
        tc.tile_pool(name="p1ps", bufs=3, space="PSUM") as pspool,
        tc.tile_pool(name="p1o", bufs=3) as opool,
    ):
        for sg in range(ST // 4):  # groups of 4 s-tiles (one h-quad)
            if sg % 2 == 0:
                xg = xpool.tile([128, CT, 1024], F32R)
                nc.scalar.dma_start(
                    out=xg, in_=x_r[:, :, sg * 512 : sg * 512 + 1024]
                )
            qt4 = opool.tile([128, 4, QKV], BF16)
            for i in range(4):
                ps = pspool.tile([128, QKV], F32)
                for ct in range(CT):
                    lhsT = xg[
                        :, ct,
                        (sg % 2) * 512 + i * 128 : (sg % 2) * 512 + (i + 1) * 128,
                    ]
                    nc.tensor.matmul(
                        out=ps[:, 0:512], lhsT=lhsT, rhs=w_sb[:, ct, 0:512],
                        start=(ct == 0), stop=False,
                    )
                    nc.tensor.matmul(
                        out=ps[:, 512:QKV], lhsT=lhsT, rhs=w_sb[:, ct, 512:QKV],
                        start=(ct == 0), stop=False,
                    )
                nc.tensor.matmul(
                    out=ps[:, 0:512], lhsT=ones1_sb, rhs=brow_sb[:, 0:512],
                    start=False, stop=True,
                )
                nc.tensor.matmul(
                    out=ps[:, 512:QKV], lhsT=ones1_sb, rhs=brow_sb[:, 512:QKV],
                    start=False, stop=True,
                )
                nc.scalar.copy(out=qt4[:, i, :], in_=ps)
            # rows (sg*4+i)*128 + w of the row-major copy -> quarter sg//8
            qi, sgq = sg // 8, sg % 8
            dstA = qkvQ[qi][sgq * 512 : (sgq + 1) * 512, :].rearrange(
                "(i p) o -> p i o", p=128
            )
            nc.scalar.dma_start(out=dstA, in_=qt4)
            # rows w*128 + (sg*4+i) of the col-major q,k copy
            dstB = qkvB2[:].rearrange("(p i) o -> p i o", i=ST)[
                :, sg * 4 : (sg + 1) * 4, :
            ]
            nc.scalar.dma_start(out=dstB, in_=qt4[:, :, 0:512])

    # ---------- phase 2: axial attention (8-tile chunks) ----------
    with (
        tc.tile_pool(name="a_qt", bufs=3) as qtpool,
        tc.tile_pool(name="a_kt", bufs=3) as ktpool,
        tc.tile_pool(name="a_vt", bufs=2) as vtpool,
        tc.tile_pool(name="a_p", bufs=3) as ppool,
        tc.tile_pool(name="a_pn", bufs=3) as pnpool,
        tc.tile_pool(name="a_rz", bufs=2) as rzpool,
        tc.tile_pool(name="a_psS", bufs=2, space="PSUM") as psumS,
        tc.tile_pool(name="a_psZ", bufs=1, space="PSUM") as psumZ,
        tc.tile_pool(name="a_psO", bufs=1, space="PSUM") as psumO,
    ):
        for branch in range(2):  # 0 = row (writes O), 1 = col (adds into O)
            for tg in range(ST // 8):  # chunks of 8 attention tiles
                if branch == 0:
                    rows = qkvQ[tg // 4][(tg % 4) * 1024 : (tg % 4 + 1) * 1024, :]
                else:
                    rows = qkvB2[tg * 1024 : (tg + 1) * 1024, :]
                vt8 = vtpool.tile([128, 8, 256], BF16)
                if branch == 0:
                    nc.gpsimd.dma_start(
                        out=vt8,
                        in_=rows[:, 512:768].rearrange("(i p) o -> p i o", p=128),
                    )
                else:
                    # v for col tiles w = tg*8+i: element (g=h, i, d) lives in
                    # quarter qi at row (h - 32*qi)*128 + w, col 512 + d
                    for qi in range(NQ):
                        src = qkvQ[qi][:].rearrange(
                            "(h w) o -> h w o", w=W
                        )[:, tg * 8 : (tg + 1) * 8, 512:768]
                        nc.gpsimd.dma_start(
                            out=vt8[32 * qi : 32 * (qi + 1), :, :], in_=src
                        )
                for hp in range(2):  # head pair
                    q8 = qtpool.tile([128, 1024], BF16)
                    nc.sync.dma_start_transpose(
                        out=q8, in_=rows[:, hp * 128 : (hp + 1) * 128]
                    )
                    k8 = ktpool.tile([128, 1024], BF16)
                    nc.sync.dma_start_transpose(
                        out=k8, in_=rows[:, 256 + hp * 128 : 256 + (hp + 1) * 128]
                    )
                    # PV output of both heads stacked into one [128, 512] bank
                    psO = [psumO.tile([128, 512], F32, name=f"psO{j}") for j in range(2)]
                    if branch == 0:
                        # hl-stacked Z: partition-offset MATMUL writes (HW-safe),
                        # then ONE full-partition recip (offset custom-DVE is not)
                        psZs = psumZ.tile([128, 1024], F32, name="psZs", tag="psZ")
                    for hl in range(2):  # head within pair
                        r0, r1 = hl * 64, (hl + 1) * 64
                        psS = psumS.tile([128, 1024], F32)
                        for i in range(8):
                            nc.tensor.matmul(
                                out=psS[:, i * 128 : (i + 1) * 128],
                                lhsT=k8[r0:r1, i * 128 : (i + 1) * 128],
                                rhs=q8[r0:r1, i * 128 : (i + 1) * 128],
                                start=True, stop=True,
                            )
                        pch = ppool.tile([128, 1024], BF16)
                        nc.scalar.activation(
                            out=pch, in_=psS, func=EXP, scale=float(SCALE)
                        )
                        if branch == 0:
                            for j in range(2):
                                nc.tensor.matmul(
                                    out=psZs[r0:r1, j * 512 : (j + 1) * 512],
                                    lhsT=ones_sb[:, 0:64],
                                    rhs=pch[:, j * 512 : (j + 1) * 512],
                                    start=True, stop=True,
                                )
                            puse = pch
                        else:
                            psZ = psumZ.tile([128, 1024], F32, tag="psZ")
                            for j in range(2):
                                nc.tensor.matmul(
                                    out=psZ[:, j * 512 : (j + 1) * 512],
                                    lhsT=ones_sb,
                                    rhs=pch[:, j * 512 : (j + 1) * 512],
                                    start=True, stop=True,
                                )
                            rz = rzpool.tile([128, 1024], F32, name="rzc")
                            nc.vector.reciprocal_approx_fast(out=rz, in_=psZ)
                            pn = pnpool.tile([128, 1024], BF16)
                            nc.vector.tensor_tensor(out=pn, in0=pch, in1=rz, op=MULT)
                            puse = pn
                        for j in range(2):  # half-chunks of 4 tiles
                            for i in range(4):
                                ii = j * 4 + i
                                nc.tensor.matmul(
                                    out=psO[j][r0:r1, i * 128 : (i + 1) * 128],
                                    lhsT=vt8[:, ii, hp * 128 + r0 : hp * 128 + r1],
                                    rhs=puse[:, ii * 128 : (ii + 1) * 128],
                                    start=True, stop=True,
                                )
                    if branch == 0:
                        rzs = rzpool.tile([128, 1024], F32)
                        nc.vector.reciprocal_approx_fast(out=rzs, in_=psZs)
                    for j in range(2):  # drain both heads at once
                        t0 = tg * 8 + j * 4  # first tile of this half
                        if branch == 0:
                            nc.vector.tensor_tensor(
                                out=O_sb[hp][:, t0 * 128 : t0 * 128 + 512],
                                in0=psO[j],
                                in1=rzs[:, j * 512 : (j + 1) * 512],
                                op=MULT,
                            )
                        else:
                            dst = O_sb[hp][:, :].rearrange(
                                "p (h w) -> p h w", w=W
                            )[:, :, t0 : t0 + 4]
                            nc.vector.tensor_tensor(
                                out=dst,
                                in0=psO[j].rearrange("p (w h) -> p h w", w=4),
                                in1=dst, op=ADD,
                            )

    # ---------- phase 3: output projection ----------
    out_r = out.ap().rearrange("(t p) s -> p t s", p=128)
    with (
        tc.tile_pool(name="f_ps", bufs=3, space="PSUM") as psumF,
        tc.tile_pool(name="f_o", bufs=3) as fpool,
    ):
        for ch in range(S // 512):
            of4 = fpool.tile([128, CT, 512], F32)
            for ot in range(CT):
                psF = psumF.tile([128, 512], F32)
                for hp in range(2):
                    nc.tensor.matmul(
                        out=psF,
                        lhsT=wout_sb[:, hp, ot * 128 : (ot + 1) * 128],
                        rhs=O_sb[hp][:, ch * 512 : (ch + 1) * 512],
                        start=(hp == 0), stop=(hp == 1),
                    )
                if ot < 2:
                    nc.scalar.activation(
                        out=of4[:, ot, :], in_=psF, func=IDENT,
                        bias=boutv[:, ot : ot + 1], scale=1.0,
                    )
                else:
                    nc.vector.tensor_scalar_add(
                        out=of4[:, ot, :], in0=psF, scalar1=boutv[:, ot : ot + 1]
                    )
            nc.gpsimd.dma_start(
                out=out_r[:, :, ch * 512 : (ch + 1) * 512], in_=of4
            )


def get_nc():
    global _CACHED_NC
    if _CACHED_NC is None:
        _CACHED_NC = build_nc()
    return _CACHED_NC


def make_in_maps(x, Wqkv, bqkv, Wout, bout):
    """Per-core input dicts: core c = (b, g) with b = c // 2, g = c % 2."""
    in_maps = []
    for c in range(8):
        b, g = c // 2, c % 2
        sel = slice(256 * g, 256 * (g + 1))
        wsel = np.concatenate(
            [Wqkv[sel, :], Wqkv[512 + 256 * g : 512 + 256 * (g + 1), :],
             Wqkv[1024 + 256 * g : 1024 + 256 * (g + 1), :]], axis=0
        )  # [768, 512]
        bsel = np.concatenate(
            [bqkv[sel], bqkv[512 + 256 * g : 512 + 256 * (g + 1)],
             bqkv[1024 + 256 * g : 1024 + 256 * (g + 1)]]
        )  # [768]
        woutT = np.ascontiguousarray(Wout[:, sel].T)  # [256, 512]
        in_maps.append(
            {
                "x": np.ascontiguousarray(x[b].reshape(C, S)),
                "wqkvT": np.ascontiguousarray(wsel.T),
                "bqkv": bsel.reshape(1, QKV).copy(),
                "ones1": np.ones((1, 128), np.float32),
                "woutT": woutT.astype(ml_dtypes.bfloat16),
                "bout": (
                    np.ascontiguousarray(bout.reshape(CT, 128).T)
                    if g == 0
                    else np.zeros((128, CT), np.float32)
                ),
            }
        )
    return in_maps


def kernel(x, Wqkv, bqkv, Wout, bout):
    x = np.asarray(x, dtype=np.float32)
    Wqkv = np.asarray(Wqkv, dtype=np.float32)
    bqkv = np.asarray(bqkv, dtype=np.float32)
    Wout = np.asarray(Wout, dtype=np.float32)
    bout = np.asarray(bout, dtype=np.float32)

    nc = get_nc()
    in_maps = make_in_maps(x, Wqkv, bqkv, Wout, bout)
    res = run_bass_kernel_spmd(nc, in_maps, core_ids=list(range(8)))
    B = x.shape[0]
    out = np.empty((B, C, H, W), dtype=np.float32)
    for b in range(B):
        acc = res.results[2 * b]["out"] + res.results[2 * b + 1]["out"]
        out[b] = acc.reshape(C, H, W)
    return out
